# revision 31
# baseline (speedup 1.0000x reference)
"""Trainium2 Bass kernel for nn_DigitCapsuleLayer (dynamic-routing capsule layer).

Strategy (v2: zero-collective, replicated routing)
--------------------------------------------------
The routing state (b, c, softmax, squash, a) is batch-global but tiny; the
per-core cost of the two AllReduces in the data-parallel scheme was dominated
by rank start-skew absorption (~50us barrier + ~23us trigger delay measured).
v2 instead REPLICATES the full-batch routing on every core: each core loads
full W plus full u in both contraction layouts (uT for s-phase, ub for
T-phase) and runs the identical full-batch routing chain; only the final
iteration computes the squash for the core's own 32-batch output slice.
No collectives, no cross-rank dependency, no skew sensitivity.

Per routing iteration (3 total; iteration 2 skips the dead b-update):
  c_ij   = exp(b) replicated over i      (ACT + PE replication matmul)
  Wc     = W * c                         (DVE, bf16, chunked under s-chain)
  s      = uT^T @ Wc  (K=9216, full B)   (PE, 2 batch-half chains)
  v      = squash(s)  = sign(s)*s^2/(1+s^2), with the softmax denominator
           applied to s columns (commutes through the matmul)
  T      = ub^T @ v   (K=256 chained)    (PE, per ri-tile, fp32 PSUM)
  a      = E^T @ (reduce_o(W*T))         (DVE mul+tree, PE group-reduce)
  b     += a                             (local; identical on every core)

All matmul operands are bf16 (fp32 PSUM accumulation); routing state fp32.
"""

import sys

sys.path.insert(0, "/opt/trn_rl_repo")

import numpy as np
import ml_dtypes

import concourse.bass as bass
import concourse.tile as tile
from concourse import mybir
from concourse.bass_utils import run_bass_kernel_spmd
from concourse.vector_clock import ScopedClock

# ----------------------------------------------------------------------------
# Walrus workarounds: this image's walrus rejects any instruction carrying
# more than one sync wait. Split Tile's tail-drain waits and any other
# multi-wait instruction into single-wait NOPs on the same engine.
# ----------------------------------------------------------------------------

_uid = [0]


def _patched_drain_and_barrier(self, tick_clock, wait_clock):
    nc = self.nc
    probe = nc.sync.nop(nofuse=True, hint="tail_drain_waits")
    wait_clock.add_sem_waits(probe.ins, ScopedClock({None: tick_clock.global_clock}))
    si = probe.ins.sync_info
    waits = list(si.on_wait) if si is not None else []
    probe.ins.sync_info = mybir.SyncInfo(on_wait=waits[:1], on_update=[])
    for w in waits[1:]:
        n = nc.sync.nop(nofuse=True, hint="tail_drain_waits")
        n.ins.sync_info = mybir.SyncInfo(on_wait=[w], on_update=[])
    nc.sync.drain()
    nc.all_engine_barrier(sem_only=True)
    assert self.sems is not None
    popped = nc._tile_sem_poison_stack.pop()
    assert popped is self._sem_poison
    nc.clear_and_free_semaphores(list(self.sems.allocated().values()))


tile.TileContext._drain_and_barrier = _patched_drain_and_barrier


def _legalize_sync_waits(nc):
    for fn in nc.m.functions:
        for bb in fn.blocks:
            insts = bb.instructions
            i = 0
            while i < len(insts):
                inst = insts[i]
                si = getattr(inst, "sync_info", None)
                waits = list(si.on_wait) if si is not None else []
                if len(waits) > 1:
                    for w in waits[:-1]:
                        _uid[0] += 1
                        nop = mybir.InstNoOp(
                            name=f"I-waitsplit-{_uid[0]}", ins=[], outs=[]
                        )
                        nop.engine = inst.engine
                        nop.sync_info = mybir.SyncInfo(on_wait=[w], on_update=[])
                        insts.insert(i, nop)
                        i += 1
                    inst.sync_info = mybir.SyncInfo(
                        on_wait=[waits[-1]], on_update=list(si.on_update)
                    )
                i += 1


# ----------------------------------------------------------------------------
# Problem constants (hardcoded per contest contract)
# ----------------------------------------------------------------------------

B, R, C, O, I = 256, 1152, 10, 16, 8
NUM_ITERS = 3
N_CORES = 8
B_LOC = B // N_CORES          # 32
HB = 2                        # batch halves of 128 (full batch on-chip)
RI = R * I                    # 9216
CO = C * O                    # 160
NT = RI // 128                # 72 ri-tiles
NCHUNK = 12                   # ri-tiles per load/scale chunk (6 chunks)
TB = 3                        # T-matmul tiles packed per PSUM bank
F32 = mybir.dt.float32
BF16 = mybir.dt.bfloat16
FP8 = mybir.dt.float8e4
bfnp = ml_dtypes.bfloat16
f8np = ml_dtypes.float8_e4m3


def _build_bass():
    nc = bass.Bass("TRN2", target_bir_lowering=False, debug=False,
                   num_devices=N_CORES)

    # DRAM I/O (per core; identical on all cores except uTo/y slice)
    Wp_d = nc.dram_tensor("Wp", [128, NT * CO], BF16, kind="ExternalInput")
    uTf_d = nc.dram_tensor("uTf", [128, NT * B], FP8, kind="ExternalInput")
    ubf_d = nc.dram_tensor("ubf", [128, HB * RI], FP8, kind="ExternalInput")
    uTo_d = nc.dram_tensor("uTo", [128, NT * B_LOC], BF16,
                           kind="ExternalInput")
    E_d = nc.dram_tensor("E", [128, 16], BF16, kind="ExternalInput")
    R8_d = nc.dram_tensor("R8", [16, 128], BF16, kind="ExternalInput")
    OA_d = nc.dram_tensor("OA", [16, 128], BF16, kind="ExternalInput")
    y_d = nc.dram_tensor("y", [B_LOC, CO], F32, kind="ExternalOutput")

    with tile.TileContext(nc) as tc:
        with (
            tc.tile_pool(name="big", bufs=1) as big,
            tc.tile_pool(name="small", bufs=1) as small,
            tc.tile_pool(name="work", bufs=2) as work,
            tc.tile_pool(name="psum", bufs=1, space="PSUM") as psum,
            tc.tile_pool(name="apsum", bufs=1, space="PSUM") as apsum,
            tc.tile_pool(name="tpsum", bufs=2, space="PSUM") as tpsum,
        ):
            # ---------------- persistent SBUF ----------------
            W_sb = big.tile([128, NT, O, C], BF16, tag="W")
            Wc_sb = big.tile([128, NT, O, C], BF16, tag="Wc")
            P_sb = big.tile([128, NT, O, C], BF16, tag="P")
            uTf_sb = big.tile([128, NT, B], FP8, tag="uTf")
            ubf_sb = big.tile([128, HB, RI], FP8, tag="ubf")
            uTo_sb = big.tile([128, NT, B_LOC], BF16, tag="uTo")
            Q_sb = big.tile([128, NT * C], BF16, tag="Q")
            H8_sb = big.tile([128, NT, 8, C], BF16, tag="H8")
            H4_sb = big.tile([128, NT, 4, C], BF16, tag="H4")
            H2_sb = big.tile([128, NT, 2, C], BF16, tag="H2")

            E_sb = small.tile([128, 16], BF16, tag="E")
            R8_sb = small.tile([16, 128], BF16, tag="R8")
            OA_sb = small.tile([16, 128], BF16, tag="OA")
            den128 = small.tile([128, C], F32, tag="den128")
            rden128 = small.tile([128, C], F32, tag="rden128")
            exp_bf = small.tile([16, NT * C], BF16, tag="exp")
            crep_bf = small.tile([128, NT * C], BF16, tag="crepbf")
            # squash scratch (full batch, one [128, CO] buffer per half)
            ss_sb = small.tile([128, HB, CO], F32, tag="ss")
            xn_sb = small.tile([128, HB, CO], F32, tag="xn")
            sg_sb = small.tile([128, HB, CO], F32, tag="sg")
            dd_sb = small.tile([128, HB, CO], F32, tag="dd")
            nm_sb = small.tile([128, HB, CO], F32, tag="nm")
            rc_sb = small.tile([128, HB, CO], F32, tag="rc")
            v_bf = small.tile([128, HB, CO], BF16, tag="v")
            # own-batch squash (iter 2)
            s2_sb = small.tile([B_LOC, CO], F32, tag="s2")
            x2_sb = small.tile([B_LOC, CO], F32, tag="x2")
            g2_sb = small.tile([B_LOC, CO], F32, tag="g2")
            d2_sb = small.tile([B_LOC, CO], F32, tag="d2")
            n2_sb = small.tile([B_LOC, CO], F32, tag="n2")
            r2_sb = small.tile([B_LOC, CO], F32, tag="r2")
            v2_sb = small.tile([B_LOC, CO], F32, tag="v2")

            # bulk loads, chunked so iter-0 matmuls can start early.
            # Order: first (W, uT) chunk, then the (squash-time) constants,
            # then remaining (W, uT) interleaved (s0-chain chases), then ub
            # (T0 chases), then uT_own (needed only at iter 2).
            Wp_v = Wp_d[:].rearrange("p (t f) -> p t f", t=NT)
            uTf_v = uTf_d[:].rearrange("p (t f) -> p t f", t=NT)
            ubf_v = ubf_d[:].rearrange("p (h r) -> p h r", h=HB)
            for ch in range(NT // NCHUNK):
                sl = slice(ch * NCHUNK, (ch + 1) * NCHUNK)
                nc.sync.dma_start(
                    out=W_sb[:, sl, :, :],
                    in_=Wp_v[:, sl, :].rearrange("p t (o c) -> p t o c", o=O))
                nc.sync.dma_start(out=uTf_sb[:, sl, :], in_=uTf_v[:, sl, :])
                if ch == 0:
                    nc.sync.dma_start(out=E_sb[:], in_=E_d[:])
                    nc.sync.dma_start(out=R8_sb[:], in_=R8_d[:])
                    nc.sync.dma_start(out=OA_sb[:], in_=OA_d[:])
            for ch in range(4):
                rsl = slice(ch * (RI // 4), (ch + 1) * (RI // 4))
                nc.sync.dma_start(out=ubf_sb[:, :, rsl], in_=ubf_v[:, :, rsl])
            nc.sync.dma_start(
                out=uTo_sb[:],
                in_=uTo_d[:].rearrange("p (t f) -> p t f", t=NT))

            def softmax_and_scale(it, a_ps):
                """b (accumulated in PSUM as a_ps) -> exp(b) -> Wc = W*exp(b);
                the softmax denominator is applied to s columns at squash time
                (it commutes through the s-matmul). The denominator chain runs
                off the critical path, overlapped with the s-chain."""
                nc.scalar.activation(exp_bf[:], a_ps[:],
                                     mybir.ActivationFunctionType.Exp)
                # replicate exp over i: crep[p=(r16*8+i), (t,c)] = exp[r16,..]
                crep_ps = tpsum.tile([128, NT * C], F32, tag="T")
                nc.tensor.matmul(crep_ps[:, 0:512], R8_sb[:], exp_bf[:, 0:512])
                nc.tensor.matmul(crep_ps[:, 512:720], R8_sb[:],
                                 exp_bf[:, 512:720])
                for k0 in range(0, NT * C, 360):
                    nc.scalar.copy(crep_bf[:, k0:k0 + 360],
                                   crep_ps[:, k0:k0 + 360])
                # Wc = W * c, chunked so the s-chain can start early
                crep_v = crep_bf[:].rearrange("p (t c) -> p t c", t=NT)
                for t0 in range(0, NT, NCHUNK):
                    t1 = t0 + NCHUNK
                    nc.vector.tensor_mul(
                        Wc_sb[:, t0:t1, :, :],
                        W_sb[:, t0:t1, :, :],
                        crep_v[:, t0:t1, :].unsqueeze(2)
                        .broadcast_to([128, t1 - t0, O, C]))
                # denominator branch (consumed only at squash time), built
                # directly in 128-row form: den128[p, (t,c)] = sum_r16 exp
                den_ps = tpsum.tile([128, NT * C], F32, tag="T")
                nc.tensor.matmul(den_ps[:, 0:512], OA_sb[:], exp_bf[:, 0:512])
                nc.tensor.matmul(den_ps[:, 512:720], OA_sb[:],
                                 exp_bf[:, 512:720])
                nc.vector.reduce_sum(
                    den128[:],
                    den_ps[:].rearrange("p (t c) -> p c t", t=NT),
                    axis=mybir.AxisListType.X)
                nc.vector.reciprocal(rden128[:], den128[:])

            def s_chain_full(it):
                """s[b, co] = uT^T @ (Wc or W) for the FULL batch, K = 9216
                chained; two batch-half chains in separate PSUM banks so
                half-0 squash can overlap the half-1 chain."""
                rhs = W_sb if it == 0 else Wc_sb
                s_ps = psum.tile([128, 1024], F32, tag="S")
                for h in range(HB):
                    for t in range(NT):
                        nc.tensor.matmul(
                            s_ps[:, h * 512:h * 512 + CO],
                            uTf_sb[:, t, h * 128:(h + 1) * 128],
                            rhs[:, t, :, :],
                            start=(t == 0), stop=(t == NT - 1))
                return s_ps

            def squash_half(s_ps, h, it):
                """v_h = s*|s|/(1+s^2) on [128, CO] fp32.
                iter 0: s = s_raw/R; iters>0: s = s_raw * rden[c]."""
                sp = s_ps[:, h * 512:h * 512 + CO]
                ss = ss_sb[:, h, :]
                xn = xn_sb[:, h, :]
                ab = sg_sb[:, h, :]
                dd = dd_sb[:, h, :]
                nm = nm_sb[:, h, :]
                rc = rc_sb[:, h, :]
                if it == 0:
                    nc.scalar.mul(ss, sp, 1.0 / R)
                else:
                    nc.vector.tensor_mul(
                        ss.rearrange("b (o c) -> b o c", o=O),
                        sp.rearrange("b (o c) -> b o c", o=O),
                        rden128[:].unsqueeze(1).broadcast_to([128, O, C]))
                # 1/(1+s^2) = sigmoid(-2*ln|s|): two ACT table ops replace
                # the (slow) DVE iterative reciprocal
                nc.scalar.activation(ab, ss,
                                     mybir.ActivationFunctionType.Abs)
                nc.vector.tensor_mul(nm, ss, ab)
                nc.scalar.activation(dd, ab,
                                     mybir.ActivationFunctionType.Ln)
                nc.scalar.activation(rc, dd,
                                     mybir.ActivationFunctionType.Sigmoid,
                                     scale=-2.0)
                nc.vector.tensor_mul(v_bf[:, h, :], nm, rc)

            def a_phase(it):
                """T = ub^T @ v per ri-tile (K=256: two batch-half chained
                matmuls; 3 tiles per PSUM bank); P = W*T batched per bank;
                Q = sum_o P; a = E^T @ Q (accumulated onto the persistent
                a_ps so b = sum of a's materializes in PSUM for free)."""
                NG = NT // 6
                for g in range(NG):
                    T_ps = tpsum.tile([128, 1024], F32, tag="T")
                    for j in range(6):
                        t = g * 6 + j
                        col = (j // 3) * 512 + (j % 3) * CO
                        nc.tensor.matmul(
                            T_ps[:, col:col + CO],
                            ubf_sb[:, 0, t * 128:(t + 1) * 128],
                            v_bf[:, 0, :], start=True, stop=False)
                        nc.tensor.matmul(
                            T_ps[:, col:col + CO],
                            ubf_sb[:, 1, t * 128:(t + 1) * 128],
                            v_bf[:, 1, :], start=False, stop=True)
                    if g % 4 > 0:
                        # ACT copies both banks in one op; the multiply then
                        # runs unit-stride bf16 at 2x on DVE
                        T_cp = work.tile([128, 2, TB * CO], BF16, tag="tcp")
                        nc.scalar.copy(
                            T_cp[:],
                            T_ps[:].rearrange("p (s q) -> p s q", s=2)
                            [:, :, 0:TB * CO])
                        nc.vector.tensor_mul(
                            P_sb[:, g * 6:(g + 1) * 6, :, :]
                            .rearrange("p (s j) o c -> p s j o c", s=2),
                            W_sb[:, g * 6:(g + 1) * 6, :, :]
                            .rearrange("p (s j) o c -> p s j o c", s=2),
                            T_cp[:].rearrange("p s (j o c) -> p s j o c",
                                              j=TB, o=O))
                    else:
                        nc.vector.tensor_mul(
                            P_sb[:, g * 6:(g + 1) * 6, :, :]
                            .rearrange("p (s j) o c -> p s j o c", s=2),
                            W_sb[:, g * 6:(g + 1) * 6, :, :]
                            .rearrange("p (s j) o c -> p s j o c", s=2),
                            T_ps[:].rearrange("p (s q) -> p s q", s=2)
                            [:, :, 0:TB * CO]
                            .rearrange("p s (j o c) -> p s j o c",
                                       j=TB, o=O))
                    if g == NG // 2 - 1:
                        # first-half o-sums overlap the second half's T/P
                        nc.vector.tensor_add(
                            H8_sb[:, 0:NT // 2, :, :],
                            P_sb[:, 0:NT // 2, 0:8, :],
                            P_sb[:, 0:NT // 2, 8:16, :])
                        nc.vector.tensor_add(
                            H4_sb[:, 0:NT // 2, :, :],
                            H8_sb[:, 0:NT // 2, 0:4, :],
                            H8_sb[:, 0:NT // 2, 4:8, :])
                    if g == 3 * NG // 4 - 1:
                        t3 = 3 * NT // 4
                        nc.vector.tensor_add(
                            H8_sb[:, NT // 2:t3, :, :],
                            P_sb[:, NT // 2:t3, 0:8, :],
                            P_sb[:, NT // 2:t3, 8:16, :])
                        nc.vector.tensor_add(
                            H4_sb[:, NT // 2:t3, :, :],
                            H8_sb[:, NT // 2:t3, 0:4, :],
                            H8_sb[:, NT // 2:t3, 4:8, :])
                        nc.vector.tensor_add(
                            H2_sb[:, 0:NT // 2, :, :],
                            H4_sb[:, 0:NT // 2, 0:2, :],
                            H4_sb[:, 0:NT // 2, 2:4, :])
                        nc.vector.tensor_add(
                            Q_sb[:].rearrange("p (t c) -> p t c", t=NT)
                            [:, 0:NT // 2, :],
                            H2_sb[:, 0:NT // 2, 0, :],
                            H2_sb[:, 0:NT // 2, 1, :])
                # sum over o: pairwise halving keeps unit-stride c-runs (2x)
                t3 = 3 * NT // 4
                nc.vector.tensor_add(H8_sb[:, t3:NT, :, :],
                                     P_sb[:, t3:NT, 0:8, :],
                                     P_sb[:, t3:NT, 8:16, :])
                nc.vector.tensor_add(H4_sb[:, t3:NT, :, :],
                                     H8_sb[:, t3:NT, 0:4, :],
                                     H8_sb[:, t3:NT, 4:8, :])
                nc.vector.tensor_add(H2_sb[:, NT // 2:NT, :, :],
                                     H4_sb[:, NT // 2:NT, 0:2, :],
                                     H4_sb[:, NT // 2:NT, 2:4, :])
                nc.vector.tensor_add(
                    Q_sb[:].rearrange("p (t c) -> p t c", t=NT)
                    [:, NT // 2:NT, :],
                    H2_sb[:, NT // 2:NT, 0, :],
                    H2_sb[:, NT // 2:NT, 1, :])
                nc.tensor.matmul(a_ps[:, 0:512], E_sb[:], Q_sb[:, 0:512],
                                 start=(it == 0), stop=True)
                nc.tensor.matmul(a_ps[:, 512:720], E_sb[:], Q_sb[:, 512:720],
                                 start=(it == 0), stop=True)

            def s_chain_own():
                """Final iteration: s only for this core's 32-batch slice
                (reuses the S-tag PSUM space)."""
                sfull = psum.tile([128, 1024], F32, tag="S")
                s2_ps = sfull[0:B_LOC, 0:CO]
                for t in range(NT):
                    nc.tensor.matmul(s2_ps, uTo_sb[:, t, :],
                                     Wc_sb[:, t, :, :],
                                     start=(t == 0), stop=(t == NT - 1))
                return s2_ps

            def squash_own(s2_ps):
                nc.vector.tensor_mul(
                    s2_sb[:].rearrange("b (o c) -> b o c", o=O),
                    s2_ps.rearrange("b (o c) -> b o c", o=O),
                    rden128[0:B_LOC, :].unsqueeze(1)
                    .broadcast_to([B_LOC, O, C]))
                nc.scalar.activation(g2_sb[:], s2_sb[:],
                                     mybir.ActivationFunctionType.Abs)
                nc.vector.tensor_mul(n2_sb[:], s2_sb[:], g2_sb[:])
                nc.scalar.activation(d2_sb[:], g2_sb[:],
                                     mybir.ActivationFunctionType.Ln)
                nc.scalar.activation(r2_sb[:], d2_sb[:],
                                     mybir.ActivationFunctionType.Sigmoid,
                                     scale=-2.0)
                nc.vector.tensor_mul(v2_sb[:], n2_sb[:], r2_sb[:])

            a_ps = apsum.tile([16, NT * C], F32, tag="A1")
            for it in range(NUM_ITERS):
                if it > 0:
                    softmax_and_scale(it, a_ps)
                if it < NUM_ITERS - 1:
                    s_ps = s_chain_full(it)
                    for h in range(HB):
                        squash_half(s_ps, h, it)
                    a_phase(it)
                else:
                    s2_ps = s_chain_own()
                    squash_own(s2_ps)
                    nc.sync.dma_start(out=y_d[:], in_=v2_sb[:])

    _legalize_sync_waits(nc)
    return nc


def _host_prep(u, W):
    """Build per-core input maps from full inputs."""
    u = np.ascontiguousarray(np.asarray(u, dtype=np.float32))
    W = np.ascontiguousarray(np.asarray(W, dtype=np.float32))

    W_perm = W[0].transpose(0, 3, 2, 1).reshape(RI, CO)          # [ri, (o,c)]
    Wp = np.ascontiguousarray(
        W_perm.reshape(NT, 128, CO).transpose(1, 0, 2).reshape(128, NT * CO)
    ).astype(bfnp)

    u_flat = u.reshape(B, RI)
    uT = u_flat.T                                                # [ri, b]
    uTf = np.ascontiguousarray(
        uT.reshape(NT, 128, B).transpose(1, 0, 2).reshape(128, NT * B)
    ).astype(f8np)
    ubf = np.ascontiguousarray(
        u_flat.reshape(HB, 128, RI).transpose(1, 0, 2).reshape(128, HB * RI)
    ).astype(f8np)

    E = np.zeros((128, 16), np.float32)
    E[np.arange(128), np.arange(128) // 8] = 1.0 / B
    E = E.astype(bfnp)
    R8 = np.zeros((16, 128), np.float32)
    R8[np.arange(128) // 8, np.arange(128)] = 1.0
    R8 = R8.astype(bfnp)
    OA = np.ones((16, 128), np.float32).astype(bfnp)

    in_maps = []
    for c in range(N_CORES):
        uTo = np.ascontiguousarray(
            uT[:, c * B_LOC:(c + 1) * B_LOC]
            .reshape(NT, 128, B_LOC).transpose(1, 0, 2)
            .reshape(128, NT * B_LOC)).astype(bfnp)
        in_maps.append({
            "Wp": Wp, "uTf": uTf, "ubf": ubf, "uTo": uTo,
            "E": E, "R8": R8, "OA": OA,
        })
    return in_maps


_cached = {}


def _get_nc():
    if "nc" not in _cached:
        _cached["nc"] = _build_bass()
    return _cached["nc"]


def kernel(u, W, _return_timing=False):
    nc = _get_nc()
    in_maps = _host_prep(u, W)
    res = run_bass_kernel_spmd(
        nc, in_maps, list(range(N_CORES)), trace=_return_timing)
    outs = [res.results[i]["y"].reshape(B_LOC, O, C).transpose(0, 2, 1)
            .reshape(B_LOC, C, O, 1) for i in range(N_CORES)]
    full = np.concatenate(outs, axis=0).astype(np.float32)
    if _return_timing:
        return full, res.exec_time_ns
    return full


# revision 32
# speedup vs baseline: 1.0764x; 1.0764x over previous
"""Trainium2 Bass kernel for nn_DigitCapsuleLayer (dynamic-routing capsule layer).

Strategy (v2: zero-collective, replicated routing)
--------------------------------------------------
The routing state (b, c, softmax, squash, a) is batch-global but tiny; the
per-core cost of the two AllReduces in the data-parallel scheme was dominated
by rank start-skew absorption (~50us barrier + ~23us trigger delay measured).
v2 instead REPLICATES the full-batch routing on every core: each core loads
full W plus full u in both contraction layouts (uT for s-phase, ub for
T-phase) and runs the identical full-batch routing chain; only the final
iteration computes the squash for the core's own 32-batch output slice.
No collectives, no cross-rank dependency, no skew sensitivity.

Per routing iteration (3 total; iteration 2 skips the dead b-update):
  c_ij   = exp(b) replicated over i      (ACT + PE replication matmul)
  Wc     = W * c                         (DVE, bf16, chunked under s-chain)
  s      = uT^T @ Wc  (K=9216, full B)   (PE, 2 batch-half chains)
  v      = squash(s)  = sign(s)*s^2/(1+s^2), with the softmax denominator
           applied to s columns (commutes through the matmul)
  T      = ub^T @ v   (K=256 chained)    (PE, per ri-tile, fp32 PSUM)
  a      = E^T @ (reduce_o(W*T))         (DVE mul+tree, PE group-reduce)
  b     += a                             (local; identical on every core)

All matmul operands are bf16 (fp32 PSUM accumulation); routing state fp32.
"""

import sys

sys.path.insert(0, "/opt/trn_rl_repo")

import numpy as np
import ml_dtypes

import concourse.bass as bass
import concourse.tile as tile
from concourse import mybir
from concourse.bass_utils import run_bass_kernel_spmd
from concourse.vector_clock import ScopedClock

# ----------------------------------------------------------------------------
# Walrus workarounds: this image's walrus rejects any instruction carrying
# more than one sync wait. Split Tile's tail-drain waits and any other
# multi-wait instruction into single-wait NOPs on the same engine.
# ----------------------------------------------------------------------------

_uid = [0]


def _patched_drain_and_barrier(self, tick_clock, wait_clock):
    nc = self.nc
    probe = nc.sync.nop(nofuse=True, hint="tail_drain_waits")
    wait_clock.add_sem_waits(probe.ins, ScopedClock({None: tick_clock.global_clock}))
    si = probe.ins.sync_info
    waits = list(si.on_wait) if si is not None else []
    probe.ins.sync_info = mybir.SyncInfo(on_wait=waits[:1], on_update=[])
    for w in waits[1:]:
        n = nc.sync.nop(nofuse=True, hint="tail_drain_waits")
        n.ins.sync_info = mybir.SyncInfo(on_wait=[w], on_update=[])
    nc.sync.drain()
    nc.all_engine_barrier(sem_only=True)
    assert self.sems is not None
    popped = nc._tile_sem_poison_stack.pop()
    assert popped is self._sem_poison
    nc.clear_and_free_semaphores(list(self.sems.allocated().values()))


tile.TileContext._drain_and_barrier = _patched_drain_and_barrier


def _legalize_sync_waits(nc):
    for fn in nc.m.functions:
        for bb in fn.blocks:
            insts = bb.instructions
            i = 0
            while i < len(insts):
                inst = insts[i]
                si = getattr(inst, "sync_info", None)
                waits = list(si.on_wait) if si is not None else []
                if len(waits) > 1:
                    for w in waits[:-1]:
                        _uid[0] += 1
                        nop = mybir.InstNoOp(
                            name=f"I-waitsplit-{_uid[0]}", ins=[], outs=[]
                        )
                        nop.engine = inst.engine
                        nop.sync_info = mybir.SyncInfo(on_wait=[w], on_update=[])
                        insts.insert(i, nop)
                        i += 1
                    inst.sync_info = mybir.SyncInfo(
                        on_wait=[waits[-1]], on_update=list(si.on_update)
                    )
                i += 1


# ----------------------------------------------------------------------------
# Problem constants (hardcoded per contest contract)
# ----------------------------------------------------------------------------

B, R, C, O, I = 256, 1152, 10, 16, 8
NUM_ITERS = 3
N_CORES = 8
B_LOC = B // N_CORES          # 32
HB = 2                        # batch halves of 128 (full batch on-chip)
RI = R * I                    # 9216
CO = C * O                    # 160
NT = RI // 128                # 72 ri-tiles
NCHUNK = 12                   # ri-tiles per load/scale chunk (6 chunks)
TB = 3                        # T-matmul tiles packed per PSUM bank
F32 = mybir.dt.float32
BF16 = mybir.dt.bfloat16
FP8 = mybir.dt.float8e4
bfnp = ml_dtypes.bfloat16
f8np = ml_dtypes.float8_e4m3


def _build_bass():
    nc = bass.Bass("TRN2", target_bir_lowering=False, debug=False,
                   num_devices=N_CORES)

    # DRAM I/O (per core; identical on all cores except uTo/y slice)
    Wp_d = nc.dram_tensor("Wp", [128, NT * CO], BF16, kind="ExternalInput")
    uTf_d = nc.dram_tensor("uTf", [128, NT * B], FP8, kind="ExternalInput")
    ubf_d = nc.dram_tensor("ubf", [128, HB * RI], FP8, kind="ExternalInput")
    uTo_d = nc.dram_tensor("uTo", [128, NT * B_LOC], BF16,
                           kind="ExternalInput")
    E_d = nc.dram_tensor("E", [128, 16], BF16, kind="ExternalInput")
    R8_d = nc.dram_tensor("R8", [16, 128], BF16, kind="ExternalInput")
    OA_d = nc.dram_tensor("OA", [16, 128], BF16, kind="ExternalInput")
    y_d = nc.dram_tensor("y", [B_LOC, CO], F32, kind="ExternalOutput")

    with tile.TileContext(nc) as tc:
        with (
            tc.tile_pool(name="big", bufs=1) as big,
            tc.tile_pool(name="small", bufs=1) as small,
            tc.tile_pool(name="work", bufs=2) as work,
            tc.tile_pool(name="psum", bufs=1, space="PSUM") as psum,
            tc.tile_pool(name="apsum", bufs=1, space="PSUM") as apsum,
            tc.tile_pool(name="tpsum", bufs=2, space="PSUM") as tpsum,
        ):
            # ---------------- persistent SBUF ----------------
            W_sb = big.tile([128, NT, O, C], BF16, tag="W")
            Wc_sb = big.tile([128, NT, O, C], BF16, tag="Wc")
            P_sb = big.tile([128, NT, O, C], BF16, tag="P")
            uTf_sb = big.tile([128, NT, B], FP8, tag="uTf")
            ubf_sb = big.tile([128, HB, RI], FP8, tag="ubf")
            uTo_sb = big.tile([128, NT, B_LOC], BF16, tag="uTo")
            Q_sb = big.tile([128, NT * C], BF16, tag="Q")
            H8_sb = big.tile([128, NT, 8, C], BF16, tag="H8")
            H4_sb = big.tile([128, NT, 4, C], BF16, tag="H4")
            H2_sb = big.tile([128, NT, 2, C], BF16, tag="H2")

            E_sb = small.tile([128, 16], BF16, tag="E")
            R8_sb = small.tile([16, 128], BF16, tag="R8")
            OA_sb = small.tile([16, 128], BF16, tag="OA")
            den128 = small.tile([128, C], F32, tag="den128")
            rden128 = small.tile([128, C], F32, tag="rden128")
            exp_bf = small.tile([16, NT * C], BF16, tag="exp")
            crep_bf = small.tile([128, NT * C], BF16, tag="crepbf")
            # squash scratch (full batch, one [128, CO] buffer per half)
            ss_sb = small.tile([128, HB, CO], F32, tag="ss")
            xn_sb = small.tile([128, HB, CO], F32, tag="xn")
            sg_sb = small.tile([128, HB, CO], F32, tag="sg")
            dd_sb = small.tile([128, HB, CO], F32, tag="dd")
            nm_sb = small.tile([128, HB, CO], F32, tag="nm")
            rc_sb = small.tile([128, HB, CO], F32, tag="rc")
            v_bf = small.tile([128, HB, CO], BF16, tag="v")
            # own-batch squash (iter 2)
            s2_sb = small.tile([B_LOC, CO], F32, tag="s2")
            x2_sb = small.tile([B_LOC, CO], F32, tag="x2")
            g2_sb = small.tile([B_LOC, CO], F32, tag="g2")
            d2_sb = small.tile([B_LOC, CO], F32, tag="d2")
            n2_sb = small.tile([B_LOC, CO], F32, tag="n2")
            r2_sb = small.tile([B_LOC, CO], F32, tag="r2")
            v2_sb = small.tile([B_LOC, CO], F32, tag="v2")

            # bulk loads, chunked so iter-0 matmuls can start early.
            # Order: first (W, uT) chunk, then the (squash-time) constants,
            # then remaining (W, uT) interleaved (s0-chain chases), then ub
            # (T0 chases), then uT_own (needed only at iter 2).
            Wp_v = Wp_d[:].rearrange("p (t f) -> p t f", t=NT)
            uTf_v = uTf_d[:].rearrange("p (t f) -> p t f", t=NT)
            ubf_v = ubf_d[:].rearrange("p (h r) -> p h r", h=HB)
            for ch in range(NT // NCHUNK):
                sl = slice(ch * NCHUNK, (ch + 1) * NCHUNK)
                nc.sync.dma_start(
                    out=W_sb[:, sl, :, :],
                    in_=Wp_v[:, sl, :].rearrange("p t (o c) -> p t o c", o=O))
                nc.sync.dma_start(out=uTf_sb[:, sl, :], in_=uTf_v[:, sl, :])
                if ch == 0:
                    nc.sync.dma_start(out=E_sb[:], in_=E_d[:])
                    nc.sync.dma_start(out=R8_sb[:], in_=R8_d[:])
                    nc.sync.dma_start(out=OA_sb[:], in_=OA_d[:])
            for ch in range(4):
                rsl = slice(ch * (RI // 4), (ch + 1) * (RI // 4))
                nc.sync.dma_start(out=ubf_sb[:, :, rsl], in_=ubf_v[:, :, rsl])
            nc.sync.dma_start(
                out=uTo_sb[:],
                in_=uTo_d[:].rearrange("p (t f) -> p t f", t=NT))

            def softmax_and_scale(it, a_ps):
                """b (accumulated in PSUM as a_ps) -> exp(b) -> Wc = W*exp(b);
                the softmax denominator is applied to s columns at squash time
                (it commutes through the s-matmul). The denominator chain runs
                off the critical path, overlapped with the s-chain."""
                nc.scalar.activation(exp_bf[:], a_ps[:],
                                     mybir.ActivationFunctionType.Exp)
                # replicate exp over i: crep[p=(r16*8+i), (t,c)] = exp[r16,..]
                crep_ps = tpsum.tile([128, NT * C], F32, tag="T")
                nc.tensor.matmul(crep_ps[:, 0:512], R8_sb[:], exp_bf[:, 0:512])
                nc.tensor.matmul(crep_ps[:, 512:720], R8_sb[:],
                                 exp_bf[:, 512:720])
                for k0 in range(0, NT * C, 360):
                    nc.scalar.copy(crep_bf[:, k0:k0 + 360],
                                   crep_ps[:, k0:k0 + 360])
                # Wc = W * c, chunked so the s-chain can start early
                crep_v = crep_bf[:].rearrange("p (t c) -> p t c", t=NT)
                for t0 in range(0, NT, NCHUNK):
                    t1 = t0 + NCHUNK
                    nc.vector.tensor_mul(
                        Wc_sb[:, t0:t1, :, :],
                        W_sb[:, t0:t1, :, :],
                        crep_v[:, t0:t1, :].unsqueeze(2)
                        .broadcast_to([128, t1 - t0, O, C]))
                # denominator branch (consumed only at squash time), built
                # directly in 128-row form: den128[p, (t,c)] = sum_r16 exp
                den_ps = tpsum.tile([128, NT * C], F32, tag="T")
                nc.tensor.matmul(den_ps[:, 0:512], OA_sb[:], exp_bf[:, 0:512])
                nc.tensor.matmul(den_ps[:, 512:720], OA_sb[:],
                                 exp_bf[:, 512:720])
                nc.vector.reduce_sum(
                    den128[:],
                    den_ps[:].rearrange("p (t c) -> p c t", t=NT),
                    axis=mybir.AxisListType.X)
                nc.vector.reciprocal(rden128[:], den128[:])

            def s_chain_full(it):
                """s[b, co] = uT^T @ (Wc or W) for the FULL batch, K = 9216
                chained; two batch-half chains in separate PSUM banks so
                half-0 squash can overlap the half-1 chain."""
                rhs = W_sb if it == 0 else Wc_sb
                s_ps = psum.tile([128, 1024], F32, tag="S")
                for h in range(HB):
                    for t in range(NT):
                        nc.tensor.matmul(
                            s_ps[:, h * 512:h * 512 + CO],
                            uTf_sb[:, t, h * 128:(h + 1) * 128],
                            rhs[:, t, :, :],
                            start=(t == 0), stop=(t == NT - 1))
                return s_ps

            def squash_half(s_ps, h, it):
                """v_h = s*|s|/(1+s^2) on [128, CO] fp32.
                iter 0: s = s_raw/R; iters>0: s = s_raw * rden[c]."""
                sp = s_ps[:, h * 512:h * 512 + CO]
                ss = ss_sb[:, h, :]
                xn = xn_sb[:, h, :]
                ab = sg_sb[:, h, :]
                dd = dd_sb[:, h, :]
                nm = nm_sb[:, h, :]
                rc = rc_sb[:, h, :]
                if it == 0:
                    nc.scalar.mul(ss, sp, 1.0 / R)
                else:
                    nc.vector.tensor_mul(
                        ss.rearrange("b (o c) -> b o c", o=O),
                        sp.rearrange("b (o c) -> b o c", o=O),
                        rden128[:].unsqueeze(1).broadcast_to([128, O, C]))
                nc.vector.tensor_mul(xn, ss, ss)
                nc.scalar.activation(ab, ss,
                                     mybir.ActivationFunctionType.Abs)
                nc.scalar.add(dd, xn, 1.0)
                nc.vector.tensor_mul(nm, ss, ab)
                nc.vector.reciprocal(rc, dd)
                nc.vector.tensor_mul(v_bf[:, h, :], nm, rc)

            def a_phase(it):
                """T = ub^T @ v per ri-tile (K=256: two batch-half chained
                matmuls; 3 tiles per PSUM bank); P = W*T batched per bank;
                Q = sum_o P; a = E^T @ Q (accumulated onto the persistent
                a_ps so b = sum of a's materializes in PSUM for free)."""
                NG = NT // 6
                for g in range(NG):
                    T_ps = tpsum.tile([128, 1024], F32, tag="T")
                    for j in range(6):
                        t = g * 6 + j
                        col = (j // 3) * 512 + (j % 3) * CO
                        nc.tensor.matmul(
                            T_ps[:, col:col + CO],
                            ubf_sb[:, 0, t * 128:(t + 1) * 128],
                            v_bf[:, 0, :], start=True, stop=False)
                        nc.tensor.matmul(
                            T_ps[:, col:col + CO],
                            ubf_sb[:, 1, t * 128:(t + 1) * 128],
                            v_bf[:, 1, :], start=False, stop=True)
                    if g % 4 > 0:
                        # ACT copies both banks in one op; the multiply then
                        # runs unit-stride bf16 at 2x on DVE
                        T_cp = work.tile([128, 2, TB * CO], BF16, tag="tcp")
                        nc.scalar.copy(
                            T_cp[:],
                            T_ps[:].rearrange("p (s q) -> p s q", s=2)
                            [:, :, 0:TB * CO])
                        nc.vector.tensor_mul(
                            P_sb[:, g * 6:(g + 1) * 6, :, :]
                            .rearrange("p (s j) o c -> p s j o c", s=2),
                            W_sb[:, g * 6:(g + 1) * 6, :, :]
                            .rearrange("p (s j) o c -> p s j o c", s=2),
                            T_cp[:].rearrange("p s (j o c) -> p s j o c",
                                              j=TB, o=O))
                    else:
                        nc.vector.tensor_mul(
                            P_sb[:, g * 6:(g + 1) * 6, :, :]
                            .rearrange("p (s j) o c -> p s j o c", s=2),
                            W_sb[:, g * 6:(g + 1) * 6, :, :]
                            .rearrange("p (s j) o c -> p s j o c", s=2),
                            T_ps[:].rearrange("p (s q) -> p s q", s=2)
                            [:, :, 0:TB * CO]
                            .rearrange("p s (j o c) -> p s j o c",
                                       j=TB, o=O))
                    if g == NG // 2 - 1:
                        # first-half o-sums overlap the second half's T/P
                        nc.vector.tensor_add(
                            H8_sb[:, 0:NT // 2, :, :],
                            P_sb[:, 0:NT // 2, 0:8, :],
                            P_sb[:, 0:NT // 2, 8:16, :])
                        nc.vector.tensor_add(
                            H4_sb[:, 0:NT // 2, :, :],
                            H8_sb[:, 0:NT // 2, 0:4, :],
                            H8_sb[:, 0:NT // 2, 4:8, :])
                    if g == 3 * NG // 4 - 1:
                        t3 = 3 * NT // 4
                        nc.vector.tensor_add(
                            H8_sb[:, NT // 2:t3, :, :],
                            P_sb[:, NT // 2:t3, 0:8, :],
                            P_sb[:, NT // 2:t3, 8:16, :])
                        nc.vector.tensor_add(
                            H4_sb[:, NT // 2:t3, :, :],
                            H8_sb[:, NT // 2:t3, 0:4, :],
                            H8_sb[:, NT // 2:t3, 4:8, :])
                        nc.vector.tensor_add(
                            H2_sb[:, 0:NT // 2, :, :],
                            H4_sb[:, 0:NT // 2, 0:2, :],
                            H4_sb[:, 0:NT // 2, 2:4, :])
                        nc.vector.tensor_add(
                            Q_sb[:].rearrange("p (t c) -> p t c", t=NT)
                            [:, 0:NT // 2, :],
                            H2_sb[:, 0:NT // 2, 0, :],
                            H2_sb[:, 0:NT // 2, 1, :])
                # sum over o: pairwise halving keeps unit-stride c-runs (2x)
                t3 = 3 * NT // 4
                nc.vector.tensor_add(H8_sb[:, t3:NT, :, :],
                                     P_sb[:, t3:NT, 0:8, :],
                                     P_sb[:, t3:NT, 8:16, :])
                nc.vector.tensor_add(H4_sb[:, t3:NT, :, :],
                                     H8_sb[:, t3:NT, 0:4, :],
                                     H8_sb[:, t3:NT, 4:8, :])
                nc.vector.tensor_add(H2_sb[:, NT // 2:NT, :, :],
                                     H4_sb[:, NT // 2:NT, 0:2, :],
                                     H4_sb[:, NT // 2:NT, 2:4, :])
                nc.vector.tensor_add(
                    Q_sb[:].rearrange("p (t c) -> p t c", t=NT)
                    [:, NT // 2:NT, :],
                    H2_sb[:, NT // 2:NT, 0, :],
                    H2_sb[:, NT // 2:NT, 1, :])
                nc.tensor.matmul(a_ps[:, 0:512], E_sb[:], Q_sb[:, 0:512],
                                 start=(it == 0), stop=True)
                nc.tensor.matmul(a_ps[:, 512:720], E_sb[:], Q_sb[:, 512:720],
                                 start=(it == 0), stop=True)

            def s_chain_own():
                """Final iteration: s only for this core's 32-batch slice
                (reuses the S-tag PSUM space)."""
                sfull = psum.tile([128, 1024], F32, tag="S")
                s2_ps = sfull[0:B_LOC, 0:CO]
                for t in range(NT):
                    nc.tensor.matmul(s2_ps, uTo_sb[:, t, :],
                                     Wc_sb[:, t, :, :],
                                     start=(t == 0), stop=(t == NT - 1))
                return s2_ps

            def squash_own(s2_ps):
                nc.vector.tensor_mul(
                    s2_sb[:].rearrange("b (o c) -> b o c", o=O),
                    s2_ps.rearrange("b (o c) -> b o c", o=O),
                    rden128[0:B_LOC, :].unsqueeze(1)
                    .broadcast_to([B_LOC, O, C]))
                nc.vector.tensor_mul(x2_sb[:], s2_sb[:], s2_sb[:])
                nc.scalar.activation(g2_sb[:], s2_sb[:],
                                     mybir.ActivationFunctionType.Abs)
                nc.scalar.add(d2_sb[:], x2_sb[:], 1.0)
                nc.vector.tensor_mul(n2_sb[:], s2_sb[:], g2_sb[:])
                nc.vector.reciprocal(r2_sb[:], d2_sb[:])
                nc.vector.tensor_mul(v2_sb[:], n2_sb[:], r2_sb[:])

            a_ps = apsum.tile([16, NT * C], F32, tag="A1")
            for it in range(NUM_ITERS):
                if it > 0:
                    softmax_and_scale(it, a_ps)
                if it < NUM_ITERS - 1:
                    s_ps = s_chain_full(it)
                    for h in range(HB):
                        squash_half(s_ps, h, it)
                    a_phase(it)
                else:
                    s2_ps = s_chain_own()
                    squash_own(s2_ps)
                    nc.sync.dma_start(out=y_d[:], in_=v2_sb[:])

    _legalize_sync_waits(nc)
    return nc


def _host_prep(u, W):
    """Build per-core input maps from full inputs."""
    u = np.ascontiguousarray(np.asarray(u, dtype=np.float32))
    W = np.ascontiguousarray(np.asarray(W, dtype=np.float32))

    W_perm = W[0].transpose(0, 3, 2, 1).reshape(RI, CO)          # [ri, (o,c)]
    Wp = np.ascontiguousarray(
        W_perm.reshape(NT, 128, CO).transpose(1, 0, 2).reshape(128, NT * CO)
    ).astype(bfnp)

    u_flat = u.reshape(B, RI)
    uT = u_flat.T                                                # [ri, b]
    uTf = np.ascontiguousarray(
        uT.reshape(NT, 128, B).transpose(1, 0, 2).reshape(128, NT * B)
    ).astype(f8np)
    ubf = np.ascontiguousarray(
        u_flat.reshape(HB, 128, RI).transpose(1, 0, 2).reshape(128, HB * RI)
    ).astype(f8np)

    E = np.zeros((128, 16), np.float32)
    E[np.arange(128), np.arange(128) // 8] = 1.0 / B
    E = E.astype(bfnp)
    R8 = np.zeros((16, 128), np.float32)
    R8[np.arange(128) // 8, np.arange(128)] = 1.0
    R8 = R8.astype(bfnp)
    OA = np.ones((16, 128), np.float32).astype(bfnp)

    in_maps = []
    for c in range(N_CORES):
        uTo = np.ascontiguousarray(
            uT[:, c * B_LOC:(c + 1) * B_LOC]
            .reshape(NT, 128, B_LOC).transpose(1, 0, 2)
            .reshape(128, NT * B_LOC)).astype(bfnp)
        in_maps.append({
            "Wp": Wp, "uTf": uTf, "ubf": ubf, "uTo": uTo,
            "E": E, "R8": R8, "OA": OA,
        })
    return in_maps


_cached = {}


def _get_nc():
    if "nc" not in _cached:
        _cached["nc"] = _build_bass()
    return _cached["nc"]


def kernel(u, W, _return_timing=False):
    nc = _get_nc()
    in_maps = _host_prep(u, W)
    res = run_bass_kernel_spmd(
        nc, in_maps, list(range(N_CORES)), trace=_return_timing)
    outs = [res.results[i]["y"].reshape(B_LOC, O, C).transpose(0, 2, 1)
            .reshape(B_LOC, C, O, 1) for i in range(N_CORES)]
    full = np.concatenate(outs, axis=0).astype(np.float32)
    if _return_timing:
        return full, res.exec_time_ns
    return full
